# revision 1
# baseline (speedup 1.0000x reference)
"""CoupledFourierSystem Trainium2 kernel.

Math: out[t,e] = sum_d W[e,d] * sum_{h,c} A[d,h,c]*cos(w[d,h,c]*s[t]+phi[d,h,c]) + b[e]

Flatten j=(d,h,c) -> 2048.  With G[j,e] = A_j * W[e,d(j)]:
    out[t,e] = sum_j cos(w_j*s_t + phi_j) * G[j,e] + b[e]
cos(x) = sin(x + pi/2);  sin(theta) computed via turns:
    u = (w_j/2pi)*s_t + p2_j      (p2 = (phi+pi/2)/2pi + 4, keeps u > 0)
    frac = u mod 1; arg = frac - 0.5        -> sin(2pi*arg) = -sin(theta)
so G is negated on the host.  Per core (t-shard of 4096):
    DVE:  u = s_b*f[p] + p2[p]          (tensor_scalar, 2 elem/cyc fp32 SBUF)
    DVE/GpSimd (alternating): arg = (u mod 1) - 0.5
    ACT:  sin(2pi*arg)                  (bottleneck: 16 ops of [128,4096])
    PE :  psum[c] += G_jt.T @ sin[:,c]  (16x8 fp32 matmuls, K=128,M=64,N=512)
Output [64, 4096] DMA'd straight from PSUM; host concatenates, transposes,
adds b.
"""
import numpy as np
from contextlib import ExitStack

import concourse.bass as bass
import concourse.tile as tile
import concourse.dve_ops as dve_ops
from concourse import mybir
from concourse.bass_utils import run_bass_kernel_spmd
from concourse.dve_table_gen import dve_ver_for
from concourse.vector_clock import ScopedClock, VectorClock

S, DIM, H = 32768, 64, 16
NCORES = 8
T = S // NCORES          # 4096 time points per core
J = DIM * H * 2          # 2048 flattened harmonics
NJT = J // 128           # 16 j-tiles
NCH = T // 512           # 8 psum chunks
f32 = mybir.dt.float32
TWO_PI = 2.0 * np.pi


# --- workaround: walrus rejects the TileContext exit drain when it carries
# >2 sem waits ("Too many sync wait commands").  Split the waits onto
# preceding SP nops (one wait each); SP is in-order so the drain still runs
# only after every outstanding proc completed.
def _split_drain_and_barrier(self, tick_clock, wait_clock):
    gc = tick_clock.global_clock
    ticks = eval(repr(gc).replace("VectorClock", ""))
    nprocs = len(ticks)
    for i, t in enumerate(ticks):
        if t == 0:
            continue
        sub = [0] * nprocs
        sub[i] = t
        nop = self.nc.sync.nop(nofuse=True, hint=f"drain_wait_p{i}")
        wait_clock.add_sem_waits(nop.ins, ScopedClock({None: VectorClock(sub)}))
    self.nc.sync.drain()
    self.nc.all_engine_barrier()
    assert self.sems is not None
    popped = self.nc._tile_sem_poison_stack.pop()
    assert popped is self._sem_poison
    self.nc.clear_and_free_semaphores(list(self.sems.allocated().values()))
    self.nc.all_engine_barrier()


tile.TileContext._drain_and_barrier = _split_drain_and_barrier

MAX_WAITS = 1
GP_JTS = 9          # j-tiles whose passes 1-2 run on GpSimd
BCAST_MODE = "dma"  # s-broadcast path: "dma" (verified) or "mm" (experimental)
MAGIC = 1.5 * 2.0 ** 23     # forces RNE-to-integer for |u| < 2^22




def _split_excess_waits(nc: bass.Bass):
    """Walrus rejects instructions carrying more than a couple of sem waits.
    Hoist excess waits onto preceding same-engine nops (engines are in-order,
    so semantics are unchanged)."""
    import copy
    m = nc.m
    new_module = copy.replace(m, functions=[])
    nid = [0]
    for function in m.functions:
        new_function = copy.replace(function, blocks=[])
        new_function.set_allocations_from_list(function.allocations)
        for block in m.functions[0].blocks if False else function.blocks:
            new_insts = []
            for inst in block.instructions:
                si = inst.sync_info
                if si is not None and len(si.on_wait) > MAX_WAITS:
                    waits = list(si.on_wait)
                    extra, keep = waits[:-MAX_WAITS], waits[-MAX_WAITS:]
                    for w_i in range(0, len(extra), MAX_WAITS):
                        nid[0] += 1
                        nop = mybir.InstNoOp(
                            name=f"{inst.name}-wsplit{nid[0]}",
                            sync_info=mybir.SyncInfo(
                                on_wait=extra[w_i:w_i + MAX_WAITS], on_update=[]
                            ),
                            bass_nofuse=True,
                            engine=inst.engine,
                        )
                        new_insts.append(nop)
                    inst.sync_info = mybir.SyncInfo(
                        on_wait=keep, on_update=list(si.on_update)
                    )
                new_insts.append(inst)
            new_block = copy.replace(block, instructions=new_insts)
            new_function.blocks.append(new_block)
        new_module.functions.append(new_function)
    nc.m = new_module


def build_nc(reps: int = 1, split_waits: bool = True) -> bass.Bass:
    nc = bass.Bass()
    s_d = nc.declare_dram_parameter("s", [T], f32, isOutput=False)
    f_d = nc.declare_dram_parameter("fv", [128, NJT], f32, isOutput=False)
    p_d = nc.declare_dram_parameter("pv", [128, NJT], f32, isOutput=False)
    g_d = nc.declare_dram_parameter("g", [128, NJT, 64], f32, isOutput=False)
    o_d = nc.declare_dram_parameter("out", [64, T], f32, isOutput=True)

    with tile.TileContext(nc) as tc, ExitStack() as ctx:
        const = ctx.enter_context(tc.tile_pool(name="const", bufs=1))
        upool = ctx.enter_context(tc.tile_pool(name="upool", bufs=2))
        kpool = ctx.enter_context(tc.tile_pool(name="kpool", bufs=2))
        apool = ctx.enter_context(tc.tile_pool(name="apool", bufs=2))
        spool = ctx.enter_context(tc.tile_pool(name="spool", bufs=3))
        psum = ctx.enter_context(tc.tile_pool(name="psum", bufs=1, space="PSUM"))

        fv_sb = const.tile([128, NJT], f32)
        pv_sb = const.tile([128, NJT], f32)
        g_sb = const.tile([128, NJT, 64], f32)
        nc.sync.dma_start(out=fv_sb, in_=f_d[:, :])
        nc.sync.dma_start(out=pv_sb, in_=p_d[:, :])
        nc.sync.dma_start(out=g_sb, in_=g_d[:, :, :])
        s_b = const.tile([128, T], f32)

        if BCAST_MODE == "mm":
            ones_sb = const.tile([1, 128], f32)
            s_row = const.tile([1, T], f32)
            nc.vector.memset(ones_sb, 1.0)
            nc.sync.dma_start(out=s_row, in_=s_d[:].reshape(1, T))

        for _ in range(reps):
            if BCAST_MODE == "mm":
                # broadcast via K=1 matmul: psum[p, t] = ones[1,p].T @ s[1,t]
                for c in range(NCH):
                    bps = psum.tile([128, 512], f32, tag=f"bc{c % 2}",
                                    name=f"bc{c}")
                    nc.tensor.matmul(
                        bps, ones_sb, s_row[:, c * 512:(c + 1) * 512],
                        start=True, stop=True,
                    )
                    nc.vector.tensor_copy(s_b[:, c * 512:(c + 1) * 512], bps)
            else:
                # broadcast s across partitions, one DMA per 512-chunk
                for c in range(NCH):
                    sl = s_d[c * 512:(c + 1) * 512]
                    bcast = bass.AP(
                        tensor=sl.tensor, offset=sl.offset,
                        ap=[[0, 128]] + [list(x) for x in sl.ap],
                    )
                    nc.sync.dma_start(out=s_b[:, c * 512:(c + 1) * 512], in_=bcast)

            psums = [
                psum.tile([64, 512], f32, tag=f"ps{c}", name=f"ps{c}")
                for c in range(NCH)
            ]
            for jt in range(NJT):
                # u = s*f + p2 ; k = rne(u) via +/- magic ; arg = u - k
                eng = nc.gpsimd if jt < GP_JTS else nc.vector
                u_t = upool.tile([128, T], f32, tag="u", name=f"u{jt}")
                eng.tensor_scalar(
                    u_t, s_b, fv_sb[:, jt:jt + 1], pv_sb[:, jt:jt + 1],
                    mybir.AluOpType.mult, mybir.AluOpType.add,
                )
                k_t = kpool.tile([128, T], f32, tag="k", name=f"k{jt}")
                eng.tensor_scalar(
                    k_t, u_t, MAGIC, MAGIC,
                    mybir.AluOpType.add, mybir.AluOpType.subtract,
                )
                a_t = apool.tile([128, T], f32, tag="a", name=f"a{jt}")
                nc.vector.tensor_tensor(
                    a_t, u_t, k_t, mybir.AluOpType.subtract
                )
                sin_t = spool.tile([128, T], f32, tag="sin", name=f"sin{jt}")
                nc.scalar.activation(
                    sin_t, a_t, mybir.ActivationFunctionType.Sin,
                    bias=0.0, scale=TWO_PI,
                )
                for c in range(NCH):
                    nc.tensor.matmul(
                        psums[c], g_sb[:, jt, :], sin_t[:, c * 512:(c + 1) * 512],
                        start=(jt == 0), stop=(jt == NJT - 1),
                    )
            for c in range(NCH):
                o_sb = spool.tile([64, 512], f32, tag="o", name=f"o{c}", bufs=4)
                nc.vector.tensor_copy(o_sb, psums[c])
                nc.sync.dma_start(
                    out=o_d[:, c * 512:(c + 1) * 512], in_=o_sb
                )
    if split_waits:
        _split_excess_waits(nc)
    return nc


def _prep_in_maps(s, A, phi, w, W):
    w_flat = np.asarray(w, np.float64).reshape(J)
    phi_flat = np.asarray(phi, np.float64).reshape(J)
    A_flat = np.asarray(A, np.float64).reshape(J)
    d_of_j = np.arange(J) // (H * 2)

    fv = (w_flat / TWO_PI).astype(np.float32).reshape(NJT, 128).T.copy()
    pv = ((phi_flat + np.pi / 2) / TWO_PI).astype(np.float32) \
        .reshape(NJT, 128).T.copy()
    G = (A_flat[:, None] * np.asarray(W, np.float64).T[d_of_j, :])
    g = G.astype(np.float32).reshape(NJT, 128, 64).transpose(1, 0, 2).copy()

    s_np = np.asarray(s, np.float32)
    return [
        {"s": s_np[i * T:(i + 1) * T].copy(), "fv": fv, "pv": pv, "g": g}
        for i in range(NCORES)
    ]


def kernel(s, x, A, phi, w, W, b):
    in_maps = _prep_in_maps(s, A, phi, w, W)
    nc = build_nc(reps=1)
    res = run_bass_kernel_spmd(nc, in_maps, core_ids=list(range(NCORES)))
    parts = [res.results[i]["out"] for i in range(NCORES)]      # each [64, T]
    full = np.concatenate(parts, axis=1).T                      # [S, 64]
    return (full + np.asarray(b, np.float32)[None, :]).astype(np.float32)

